# revision 8
# baseline (speedup 1.0000x reference)
"""Depthwise 5x5 box filter (stride 1, 'same' zero padding) on TRN2.

Input x: (16, 8, 512, 512) f32, weight: (1, 1, 5, 5) f32 (uniform box kernel).
Output: (16, 8, 512, 512) f32.

Strategy
--------
Data-parallel over the 128 independent (n, c) planes: 16 planes per core
across 8 cores.  Per plane, the separable 5-tap box filter runs entirely on
the TensorEngine as two "transposing" banded matmuls:

  pass A:  mid[w, h'] = sum_h  img[h, w] * Band[h, h']   (vertical 5-sum)
  pass B:  out[h, w'] = sum_w  mid[w, h'] * Band[w, w']  (horizontal 5-sum)

Each pass contracts over the partition dimension of its input, so the
output of each matmul comes out transposed — two passes restore the
original orientation with no explicit transpose ops.  Band is a 0/1
banded Toeplitz matrix (values exactly representable), the final x(1/25)
scale is folded into the pass-B PSUM->SBUF copies.

Contraction over a full 512-row dimension is tiled into 4 K-blocks of
128; their overlapping 130/132-wide output windows accumulate in one
PSUM bank using the per-element has_written mechanism (verified on HW).

Host-side, the image is cast to fp16 (and results returned from fp16):
halves DMA traffic, and fp16 matmuls stream at 1 column/cycle on the PE.
HBM layouts are host-packed so DMAs have contiguous multi-KiB partition
lines:
  input  xs[g, p, pl2, hb, w]  (2 planes per DMA, p = h % 128)
  output ys[pl, p, hb, w]      (1 plane per DMA; pass B restores
                                orientation so packing matches input)

The kernel is HBM-bound: 16.8 MB of traffic at ~358 GB/s/core ≈ 47 us.
The schedule aims to keep HBM saturated end-to-end:
  - all 16 input planes are SBUF-buffered (img bufs=4 x 2-plane tiles x
    8 KiB/partition), so input DMAs are issued greedily with no
    compute-side gating;
  - PSUM->SBUF copies are the per-plane compute cadence limiter; they
    are split evenly: ScalarE takes PSUM banks 0-1 ([128,1024], ~1.07us),
    VectorE banks 2-3 (one [128,1024] op), per pass, so both engines sit
    at ~2.2us/plane, under the 2.93us/plane HBM share;
  - output DMAs go per-plane on GpSimd/SWDGE (4 queues) so the sync
    (input) HWDGE ring is never blocked behind a copy dependency.
"""

from contextlib import ExitStack

import numpy as np

import concourse.bacc as bacc
import concourse.tile as tile
from concourse import mybir
from concourse.bass_utils import run_bass_kernel_spmd

N_CORES = 8
PLANES_TOTAL = 128  # 16 batch * 8 channels
PLANES_PER_CORE = PLANES_TOTAL // N_CORES  # 16
H = W = 512
P = 128  # partitions / K-block
NB = P + 4  # band matrix columns
KTAP = 5
KPAD = 2

MM_DT = mybir.dt.float16
NP_IO_DT = np.float16

# Per PSUM bank (one 512-wide output window) the 4 K-block matmuls write
# overlapping band windows; the first (start=True) clears the whole-bank
# pending-zero region, and subsequent matmuls accumulate where written /
# overwrite where pending, per-element (PSUM has_written semantics).
# (kb, out_lo, out_hi, band_lo, band_hi, start)
BANK_PLAN = [
    (0, 0, 130, 2, 132, True),
    (1, 126, 258, 0, 132, False),
    (2, 254, 386, 0, 132, False),
    (3, 382, 512, 0, 130, False),
]


def _band_host() -> np.ndarray:
    """B[p, j] = 1.0 iff 0 <= j - p <= 4, shape [128, 132]."""
    b = np.zeros((P, NB), dtype=np.float32)
    for p in range(P):
        b[p, p : p + KTAP] = 1.0
    return b.astype(np.float16)


def _emit_bank(nc, ps, band, lhsT_of, last_bank):
    for i, (kb, o0, o1, b0, b1, start) in enumerate(BANK_PLAN):
        nc.tensor.matmul(
            ps[:, o0:o1],
            lhsT_of(kb),
            band[:, b0:b1],
            start=start,
            stop=(last_bank and i == len(BANK_PLAN) - 1),
        )


def _build_nc(scale: float):
    nc = bacc.Bacc("TRN2", num_devices=N_CORES, num_swdge_queues=4)
    # 2 planes per input DMA; HBM layout packed so each partition line is
    # one contiguous 8 KiB chunk.
    xs = nc.declare_dram_parameter(
        "xs", [PLANES_PER_CORE // 2, P, 2, 4, W], MM_DT, isOutput=False
    )
    band_d = nc.declare_dram_parameter("band", [P, NB], MM_DT, isOutput=False)
    ys = nc.declare_dram_parameter(
        "ys", [PLANES_PER_CORE, P, 4, W], MM_DT, isOutput=True
    )

    with ExitStack() as ctx:
        tc = ctx.enter_context(tile.TileContext(nc))
        const_pool = ctx.enter_context(tc.tile_pool(name="const", bufs=1))
        # 8 bufs x 2-plane tiles = all 16 planes bufferable, so input DMAs
        # are never gated by compute progress.
        img_pool = ctx.enter_context(tc.tile_pool(name="img", bufs=8))
        mid_pool = ctx.enter_context(tc.tile_pool(name="mid", bufs=4))
        out_pool = ctx.enter_context(tc.tile_pool(name="out", bufs=6))
        psa_pool = ctx.enter_context(tc.tile_pool(name="psa", bufs=1, space="PSUM"))
        psb_pool = ctx.enter_context(tc.tile_pool(name="psb", bufs=1, space="PSUM"))

        band = const_pool.tile([P, NB], MM_DT, tag="band")
        nc.sync.dma_start(band[:], band_d[:])

        def emit_load(g):
            # One dense input DMA per 2 planes on the sync HWDGE ring.
            img = img_pool.tile([P, 2 * 4 * W], MM_DT, tag="img", name=f"img{g}")
            nc.sync.dma_start(
                img[:].rearrange("p (g b w) -> p g b w", b=4, w=W),
                xs[g],
            )
            return img

        def emit_store(pl, out2):
            # One dense output DMA per plane on the SWDGE queues.
            nc.gpsimd.dma_start(
                ys[pl],
                out2[:].rearrange("p (b w) -> p b w", w=W),
            )

        # Software pipeline, one plane deep: the PE stream interleaves
        # pass A of plane pl with pass B of plane pl-1 at bank
        # granularity, so the PE never sits behind the PSUM->SBUF copies.
        LAG = 1
        mids, outs = {}, {}
        imgs = {0: emit_load(0)}
        mids[0] = mid_pool.tile([P, 4 * W], MM_DT, tag="mid", name="mid0")
        for pl in range(PLANES_PER_CORE + LAG):
            g, sub = pl // 2, pl % 2
            if pl + 1 < PLANES_PER_CORE and (pl + 1) % 2 == 0:
                imgs[(pl + 1) // 2] = emit_load((pl + 1) // 2)
            bp = pl - LAG
            if bp >= 0:
                outs[bp] = out_pool.tile(
                    [P, 4 * W], MM_DT, tag="out", name=f"out{bp}"
                )
            if pl < PLANES_PER_CORE:
                psa = psa_pool.tile(
                    [P, 4 * W], mybir.dt.float32, tag="psa", name=f"psa{pl}"
                )
            if bp >= 0:
                psb = psb_pool.tile(
                    [P, 4 * W], mybir.dt.float32, tag="psb", name=f"psb{bp}"
                )
            # Interleave A(pl) and B(bp) banks so the PE alternates and
            # both PSUM tiles fill concurrently.
            for b in range(4):
                if pl < PLANES_PER_CORE:
                    _emit_bank(
                        nc,
                        psa[:, b * W : (b + 1) * W],
                        band,
                        lambda kb, b=b: imgs[g][
                            :, sub * 4 * W + kb * W + b * P : sub * 4 * W + kb * W + (b + 1) * P
                        ],
                        last_bank=True,
                    )
                if bp >= 0:
                    _emit_bank(
                        nc,
                        psb[:, b * W : (b + 1) * W],
                        band,
                        lambda kb, b=b: mids[bp][
                            :, kb * W + b * P : kb * W + (b + 1) * P
                        ],
                        last_bank=True,
                    )
            # Balanced PSUM->SBUF copies: ScalarE takes banks 0-1,
            # VectorE banks 2-3 (single wide op), per pass.
            if pl < PLANES_PER_CORE:
                nc.scalar.copy(mids[pl][:, 0 : 2 * W], psa[:, 0 : 2 * W])
                nc.vector.tensor_copy(
                    mids[pl][:, 2 * W : 4 * W], psa[:, 2 * W : 4 * W]
                )
            if bp >= 0:
                nc.scalar.mul(outs[bp][:, 0 : 2 * W], psb[:, 0 : 2 * W], scale)
                nc.vector.tensor_scalar_mul(
                    outs[bp][:, 2 * W : 4 * W], psb[:, 2 * W : 4 * W], scale
                )
                emit_store(bp, outs[bp])
            if pl + 1 < PLANES_PER_CORE:
                mids[pl + 1] = mid_pool.tile(
                    [P, 4 * W], MM_DT, tag="mid", name=f"mid{pl + 1}"
                )

    nc.compile()
    return nc


_CACHE: dict = {}


def _get_nc(scale: float):
    if scale not in _CACHE:
        _CACHE[scale] = _build_nc(scale)
    return _CACHE[scale]


def kernel(x: np.ndarray, weight: np.ndarray, _trace: bool = False):
    x = np.ascontiguousarray(x, dtype=np.float32)
    w = np.asarray(weight, dtype=np.float32).reshape(KTAP, KTAP)
    scale = float(w[KPAD, KPAD])  # 1/25 for the box kernel

    # Host-pack: [pl, h, w] -> [g, p, pl2, hb, w] with pl = 2g + pl2 and
    # h = hb*128 + p, so each partition line of a 2-plane input DMA is
    # one contiguous 8 KiB HBM chunk.
    xs = (
        x.reshape(PLANES_TOTAL // 2, 2, 4, P, W)
        .transpose(0, 3, 1, 2, 4)
        .astype(NP_IO_DT)
    )
    xs = np.ascontiguousarray(xs)
    band = _band_host()

    gpc = PLANES_PER_CORE // 2
    nc = _get_nc(scale)
    in_maps = [
        {
            "xs": xs[k * gpc : (k + 1) * gpc],
            "band": band,
        }
        for k in range(N_CORES)
    ]
    res = run_bass_kernel_spmd(nc, in_maps, list(range(N_CORES)), trace=_trace)
    # ys[pl, p, hb, w] holds out[h = hb*128 + p, w]: invert on host.
    ys = np.concatenate(
        [np.asarray(r["ys"], dtype=np.float32) for r in res.results], axis=0
    )
    out = ys.transpose(0, 2, 1, 3).reshape(PLANES_TOTAL, H, W)
    if _trace:
        kernel.last_exec_time_ns = res.exec_time_ns
    return np.ascontiguousarray(out).reshape(16, 8, H, W)


# revision 9
# speedup vs baseline: 1.3455x; 1.3455x over previous
"""Depthwise 5x5 box filter (stride 1, 'same' zero padding) on TRN2.

Input x: (16, 8, 512, 512) f32, weight: (1, 1, 5, 5) f32 (uniform box kernel).
Output: (16, 8, 512, 512) f32.

Strategy
--------
Data-parallel over the 128 independent (n, c) planes: 16 planes per core
across 8 cores.  Per plane, the separable 5-tap box filter runs entirely on
the TensorEngine as two "transposing" banded matmuls:

  pass A:  mid[w, h'] = sum_h  img[h, w] * Band[h, h']   (vertical 5-sum)
  pass B:  out[h, w'] = sum_w  mid[w, h'] * Band[w, w']  (horizontal 5-sum)

Each pass contracts over the partition dimension of its input, so the
output of each matmul comes out transposed — two passes restore the
original orientation with no explicit transpose ops.  Band is a 0/1
banded Toeplitz matrix (values exactly representable), the final x(1/25)
scale is folded into the pass-B PSUM->SBUF copies.

Contraction over a full 512-row dimension is tiled into 4 K-blocks of
128; their overlapping 130/132-wide output windows accumulate in one
PSUM bank using the per-element has_written mechanism (verified on HW).

Host-side, the image is cast to fp16 (and results returned from fp16):
halves DMA traffic, and fp16 matmuls stream at 1 column/cycle on the PE.
HBM layouts are host-packed so DMAs have contiguous multi-KiB partition
lines:
  input  xs[g, p, pl2, hb, w]  (2 planes per DMA, p = h % 128)
  output ys[pl, p, hb, w]      (1 plane per DMA; pass B restores
                                orientation so packing matches input)

The kernel is HBM-bound: 16.8 MB of traffic at ~358 GB/s/core ≈ 47 us.
Scheduling decisions that keep HBM saturated end-to-end:
  - ALL input DMAs are HWDGE on the *scalar* (ACT) ring, issued up-front
    before any ACT copy work, into an 8 x 2-plane img pool (all 16
    planes SBUF-resident), so the input stream runs at full rate with
    no compute-side gating and no FIFO entry ever blocks another.
  - Output DMAs are HWDGE on the *sync* (SP) ring — a different physical
    ring, so input and output transfers round-robin fairly.  SWDGE
    (GpSimd) is deliberately NOT used for outputs: VectorE copy /
    tensor_scalar ops enter 2-port SBUF perf modes that lock GpSimd out
    of its shared SBUF port and starve SWDGE descriptor generation
    (observed: output stream capped at ~140 GB/s).  HWDGE descriptor
    generation is RTL and immune.
  - PSUM->SBUF copies are the compute-cadence limiter (~2.2-2.5
    us/plane/engine): per pass, ScalarE copies PSUM banks 0-1 as one
    [128,1024] op, VectorE copies banks 2-3 as two [128,512] ops
    (VectorE pays a bank-crossing penalty on wider PSUM reads).  Copies
    are emitted per-bank so they start as soon as their accumulation
    group finishes, keeping the PE from stalling on PSUM reuse (a single
    4-bank PSUM tile per pass was measured 1.2 us/plane slower).
"""

from contextlib import ExitStack

import numpy as np

import concourse.bacc as bacc
import concourse.tile as tile
from concourse import mybir
from concourse.bass_utils import run_bass_kernel_spmd

N_CORES = 8
PLANES_TOTAL = 128  # 16 batch * 8 channels
PLANES_PER_CORE = PLANES_TOTAL // N_CORES  # 16
H = W = 512
P = 128  # partitions / K-block
NB = P + 4  # band matrix columns
KTAP = 5
KPAD = 2

MM_DT = mybir.dt.float16
NP_IO_DT = np.float16

# Per PSUM bank (one 512-wide output window) the 4 K-block matmuls write
# overlapping band windows; the first (start=True) clears the whole-bank
# pending-zero region, and subsequent matmuls accumulate where written /
# overwrite where pending, per-element (PSUM has_written semantics).
# (kb, out_lo, out_hi, band_lo, band_hi, start)
BANK_PLAN = [
    (0, 0, 130, 2, 132, True),
    (1, 126, 258, 0, 132, False),
    (2, 254, 386, 0, 132, False),
    (3, 382, 512, 0, 130, False),
]


def _band_host() -> np.ndarray:
    """B[p, j] = 1.0 iff 0 <= j - p <= 4, shape [128, 132]."""
    b = np.zeros((P, NB), dtype=np.float32)
    for p in range(P):
        b[p, p : p + KTAP] = 1.0
    return b.astype(np.float16)


def _emit_bank(nc, ps, band, lhsT_of, last_bank):
    for i, (kb, o0, o1, b0, b1, start) in enumerate(BANK_PLAN):
        nc.tensor.matmul(
            ps[:, o0:o1],
            lhsT_of(kb),
            band[:, b0:b1],
            start=start,
            stop=(last_bank and i == len(BANK_PLAN) - 1),
        )


def _build_nc(scale: float):
    nc = bacc.Bacc("TRN2", num_devices=N_CORES, num_swdge_queues=1)
    NG = PLANES_PER_CORE // 2  # 2 planes per input DMA
    xs = nc.declare_dram_parameter("xs", [NG, P, 2, 4, W], MM_DT, isOutput=False)
    band_d = nc.declare_dram_parameter("band", [P, NB], MM_DT, isOutput=False)
    ys = nc.declare_dram_parameter(
        "ys", [PLANES_PER_CORE, P, 4, W], MM_DT, isOutput=True
    )

    with ExitStack() as ctx:
        tc = ctx.enter_context(tile.TileContext(nc))
        const_pool = ctx.enter_context(tc.tile_pool(name="const", bufs=1))
        img_pool = ctx.enter_context(tc.tile_pool(name="img", bufs=NG))
        mid_pool = ctx.enter_context(tc.tile_pool(name="mid", bufs=4))
        out_pool = ctx.enter_context(tc.tile_pool(name="out", bufs=8))
        psa_pool = ctx.enter_context(tc.tile_pool(name="psa", bufs=1, space="PSUM"))
        psb_pool = ctx.enter_context(tc.tile_pool(name="psb", bufs=1, space="PSUM"))

        band = const_pool.tile([P, NB], MM_DT, tag="band")
        nc.scalar.dma_start(band[:], band_d[:])

        # All input DMAs up-front on the scalar HWDGE ring: every plane
        # has a buffer, so none of these issues ever blocks, and the ACT
        # sequencer is done issuing before its first PSUM copy is needed.
        imgs = {}
        for g in range(NG):
            img = img_pool.tile([P, 2 * 4 * W], MM_DT, tag="img", name=f"img{g}")
            nc.scalar.dma_start(
                img[:].rearrange("p (g b w) -> p g b w", b=4, w=W),
                xs[g],
            )
            imgs[g] = img

        def emit_a_bank(pl, wb, pair_ps):
            # pass A bank: mid[:, wb] = vertical 5-sum of img, transposed.
            img2, sub = imgs[pl // 2], pl % 2
            base = sub * 4 * W
            if wb == 0:
                ps = pair_ps["a"] = psa_pool.tile(
                    [P, 2 * W], mybir.dt.float32, tag="psa", name=f"psa{pl}_01"
                )
            if wb in (0, 1):
                ps = pair_ps["a"]
                view = ps[:, wb * W : (wb + 1) * W]
            else:
                ps = psa_pool.tile(
                    [P, W], mybir.dt.float32, tag="psa1", name=f"psa{pl}_{wb}",
                    bufs=2,
                )
                view = ps[:]
            _emit_bank(
                nc,
                view,
                band,
                lambda kb: img2[:, base + kb * W + wb * P : base + kb * W + (wb + 1) * P],
                last_bank=True,
            )
            if wb == 1:
                nc.scalar.copy(mids[pl][:, 0 : 2 * W], ps[:])
            elif wb in (2, 3):
                nc.vector.tensor_copy(mids[pl][:, wb * W : (wb + 1) * W], view)

        def emit_b_bank(pl, mid, out2, hb2, pair_ps):
            # pass B bank: out2[:, hb2] = horizontal 5-sum of mid, transposed
            if hb2 == 0:
                ps = pair_ps["b"] = psb_pool.tile(
                    [P, 2 * W], mybir.dt.float32, tag="psb", name=f"psb{pl}_01"
                )
            if hb2 in (0, 1):
                ps = pair_ps["b"]
                view = ps[:, hb2 * W : (hb2 + 1) * W]
            else:
                ps = psb_pool.tile(
                    [P, W], mybir.dt.float32, tag="psb1", name=f"psb{pl}_{hb2}",
                    bufs=2,
                )
                view = ps[:]
            _emit_bank(
                nc,
                view,
                band,
                lambda kb: mid[:, kb * W + hb2 * P : kb * W + (hb2 + 1) * P],
                last_bank=True,
            )
            if hb2 == 1:
                nc.scalar.mul(out2[:, 0 : 2 * W], ps[:], scale)
            elif hb2 in (2, 3):
                nc.vector.tensor_scalar_mul(
                    out2[:, hb2 * W : (hb2 + 1) * W], view, scale
                )

        def emit_store(pl, out2):
            # One dense output DMA per plane on the sync HWDGE ring.
            nc.sync.dma_start(
                ys[pl],
                out2[:].rearrange("p (b w) -> p b w", w=W),
            )

        # Software pipeline, LAG planes deep: the PE stream interleaves
        # pass A of plane pl with pass B of plane pl-LAG at bank
        # granularity, so the PE never sits behind the PSUM->SBUF copies
        # it just queued.
        LAG = 1
        mids, outs = {}, {}
        mids[0] = mid_pool.tile([P, 4 * W], MM_DT, tag="mid", name="mid0")
        for pl in range(PLANES_PER_CORE + LAG):
            bp = pl - LAG
            if bp >= 0:
                outs[bp] = out_pool.tile(
                    [P, 4 * W], MM_DT, tag="out", name=f"out{bp}"
                )
            pair_ps = {}
            for b in range(4):
                if pl < PLANES_PER_CORE:
                    emit_a_bank(pl, b, pair_ps)
                if bp >= 0:
                    emit_b_bank(bp, mids[bp], outs[bp], b, pair_ps)
            if bp >= 0:
                emit_store(bp, outs[bp])
            if pl + 1 < PLANES_PER_CORE:
                mids[pl + 1] = mid_pool.tile(
                    [P, 4 * W], MM_DT, tag="mid", name=f"mid{pl + 1}"
                )

    nc.compile()
    return nc


_CACHE: dict = {}


def _get_nc(scale: float):
    if scale not in _CACHE:
        _CACHE[scale] = _build_nc(scale)
    return _CACHE[scale]


def kernel(x: np.ndarray, weight: np.ndarray, _trace: bool = False):
    x = np.ascontiguousarray(x, dtype=np.float32)
    w = np.asarray(weight, dtype=np.float32).reshape(KTAP, KTAP)
    scale = float(w[KPAD, KPAD])  # 1/25 for the box kernel

    # Host-pack: [pl, h, w] -> [g, p, pl2, hb, w] with pl = 2g + pl2 and
    # h = hb*128 + p, so each partition line of a 2-plane input DMA is
    # one contiguous 8 KiB HBM chunk.
    xs = (
        x.reshape(PLANES_TOTAL // 2, 2, 4, P, W)
        .transpose(0, 3, 1, 2, 4)
        .astype(NP_IO_DT)
    )
    xs = np.ascontiguousarray(xs)
    band = _band_host()

    gpc = PLANES_PER_CORE // 2
    nc = _get_nc(scale)
    in_maps = [
        {
            "xs": xs[k * gpc : (k + 1) * gpc],
            "band": band,
        }
        for k in range(N_CORES)
    ]
    res = run_bass_kernel_spmd(nc, in_maps, list(range(N_CORES)), trace=_trace)
    # ys[pl, p, hb, w] holds out[h = hb*128 + p, w]: invert on host.
    ys = np.concatenate(
        [np.asarray(r["ys"], dtype=np.float32) for r in res.results], axis=0
    )
    out = ys.transpose(0, 2, 1, 3).reshape(PLANES_TOTAL, H, W)
    if _trace:
        kernel.last_exec_time_ns = res.exec_time_ns
    return np.ascontiguousarray(out).reshape(16, 8, H, W)


# revision 18
# speedup vs baseline: 1.3518x; 1.0046x over previous
"""Depthwise 5x5 box filter (stride 1, 'same' zero padding) on TRN2.

Input x: (16, 8, 512, 512) f32, weight: (1, 1, 5, 5) f32 (uniform box kernel).
Output: (16, 8, 512, 512) f32.

Strategy
--------
Data-parallel over the 128 independent (n, c) planes: 16 planes per core
across 8 cores.  Per plane, the separable 5-tap box filter runs entirely on
the TensorEngine as two "transposing" banded matmuls:

  pass A:  mid[w, h'] = sum_h  img[h, w] * Band[h, h']   (vertical 5-sum)
  pass B:  out[h, w'] = sum_w  mid[w, h'] * Band[w, w']  (horizontal 5-sum)

Each pass contracts over the partition dimension of its input, so the
output of each matmul comes out transposed — two passes restore the
original orientation with no explicit transpose ops.  Band is a 0/1
banded Toeplitz matrix (values exactly representable), the final x(1/25)
scale is folded into the pass-B PSUM->SBUF copies.

Contraction over a full 512-row dimension is tiled into 4 K-blocks of
128; their overlapping 130/132-wide output windows accumulate in one
PSUM bank using the per-element has_written mechanism (verified on HW).

Host-side, the image is cast to fp16 (and results returned from fp16):
halves DMA traffic, and fp16 matmuls stream at 1 column/cycle on the PE.
HBM layouts are host-packed so DMAs have contiguous multi-KiB partition
lines:
  input  xs[pl, p, hb, w]  (p = h % 128; planes 0-1 get single-plane
                            DMAs for fast pipeline start, then 2/DMA)
  output ys[pl, p, hb, w]  (1 plane per DMA; pass B restores
                            orientation so packing matches input)

The kernel is HBM-bound: 16.8 MB of traffic at ~358 GB/s/core ≈ 47 us.
Scheduling decisions that keep HBM saturated end-to-end:
  - ALL input DMAs are HWDGE on the *scalar* (ACT) ring, issued up-front
    before any ACT copy work, into an 8 x 2-plane img pool (all 16
    planes SBUF-resident), so the input stream runs at full rate with
    no compute-side gating and no FIFO entry ever blocks another.
  - Output DMAs are HWDGE on the *sync* (SP) ring — a different physical
    ring, so input and output transfers round-robin fairly.  SWDGE
    (GpSimd) is deliberately NOT used for outputs: VectorE copy /
    tensor_scalar ops enter 2-port SBUF perf modes that lock GpSimd out
    of its shared SBUF port and starve SWDGE descriptor generation
    (observed: output stream capped at ~140 GB/s).  HWDGE descriptor
    generation is RTL and immune.
  - PSUM->SBUF copies are the compute-cadence limiter (~2.2-2.5
    us/plane/engine): per pass, ScalarE copies PSUM banks 0-1 as one
    [128,1024] op, VectorE copies banks 2-3 as two [128,512] ops
    (VectorE pays a bank-crossing penalty on wider PSUM reads).  Copies
    are emitted per-bank so they start as soon as their accumulation
    group finishes, keeping the PE from stalling on PSUM reuse (a single
    4-bank PSUM tile per pass was measured 1.2 us/plane slower).
"""

from contextlib import ExitStack

import numpy as np

import concourse.bacc as bacc
import concourse.tile as tile
from concourse import mybir
from concourse.bass_utils import run_bass_kernel_spmd

N_CORES = 8
PLANES_TOTAL = 128  # 16 batch * 8 channels
PLANES_PER_CORE = PLANES_TOTAL // N_CORES  # 16
H = W = 512
P = 128  # partitions / K-block
NB = P + 4  # band matrix columns
KTAP = 5
KPAD = 2

MM_DT = mybir.dt.float16
NP_IO_DT = np.float16

# Per PSUM bank (one 512-wide output window) the 4 K-block matmuls write
# overlapping band windows; the first (start=True) clears the whole-bank
# pending-zero region, and subsequent matmuls accumulate where written /
# overwrite where pending, per-element (PSUM has_written semantics).
# (kb, out_lo, out_hi, band_lo, band_hi, start)
BANK_PLAN = [
    (0, 0, 130, 2, 132, True),
    (1, 126, 258, 0, 132, False),
    (2, 254, 386, 0, 132, False),
    (3, 382, 512, 0, 130, False),
]


def _band_host() -> np.ndarray:
    """B[p, j] = 1.0 iff 0 <= j - p <= 4, shape [128, 132]."""
    b = np.zeros((P, NB), dtype=np.float32)
    for p in range(P):
        b[p, p : p + KTAP] = 1.0
    return b.astype(np.float16)


def _emit_bank(nc, ps, band, lhsT_of, last_bank):
    for i, (kb, o0, o1, b0, b1, start) in enumerate(BANK_PLAN):
        nc.tensor.matmul(
            ps[:, o0:o1],
            lhsT_of(kb),
            band[:, b0:b1],
            start=start,
            stop=(last_bank and i == len(BANK_PLAN) - 1),
        )


def _build_nc(scale: float):
    nc = bacc.Bacc("TRN2", num_devices=N_CORES, num_swdge_queues=1)
    xs = nc.declare_dram_parameter(
        "xs", [PLANES_PER_CORE, P, 4, W], MM_DT, isOutput=False
    )
    band_d = nc.declare_dram_parameter("band", [P, NB], MM_DT, isOutput=False)
    ys = nc.declare_dram_parameter(
        "ys", [PLANES_PER_CORE, P, 4, W], MM_DT, isOutput=True
    )

    with ExitStack() as ctx:
        tc = ctx.enter_context(tile.TileContext(nc))
        const_pool = ctx.enter_context(tc.tile_pool(name="const", bufs=1))
        img_pool = ctx.enter_context(tc.tile_pool(name="img", bufs=2))
        img2_pool = ctx.enter_context(tc.tile_pool(name="img2", bufs=7))
        mid_pool = ctx.enter_context(tc.tile_pool(name="mid", bufs=4))
        out_pool = ctx.enter_context(tc.tile_pool(name="out", bufs=8))
        psa_pool = ctx.enter_context(tc.tile_pool(name="psa", bufs=1, space="PSUM"))
        psb_pool = ctx.enter_context(tc.tile_pool(name="psb", bufs=1, space="PSUM"))

        band = const_pool.tile([P, NB], MM_DT, tag="band")
        nc.scalar.dma_start(band[:], band_d[:])

        # All input DMAs up-front on the scalar HWDGE ring: every plane
        # has a buffer, so none of these issues ever blocks, and the ACT
        # sequencer is done issuing before its first PSUM copy is needed.
        # Planes 0 and 1 get their own single-plane DMAs so the first
        # pass-A matmuls start ~1.5us earlier; the rest go 2 planes per
        # DMA to halve issue overhead.
        groups = [[0], [1]] + [[i, i + 1] for i in range(2, PLANES_PER_CORE, 2)]
        plane_view = {}
        for gi, grp in enumerate(groups):
            n = len(grp)
            pool = img_pool if n == 1 else img2_pool
            img = pool.tile(
                [P, n * 4 * W], MM_DT, tag=f"img{n}", name=f"img_g{gi}"
            )
            nc.scalar.dma_start(
                img[:].rearrange("p (g b w) -> p g b w", b=4, w=W),
                xs[grp[0] : grp[0] + n].rearrange("g p b w -> p g b w"),
            )
            for j, pl in enumerate(grp):
                plane_view[pl] = img[:, j * 4 * W : (j + 1) * 4 * W]

        def emit_a_bank(pl, wb, pair_ps):
            # pass A bank: mid[:, wb] = vertical 5-sum of img, transposed.
            # Banks 0,1 accumulate in one 2-bank psum tile copied by
            # ScalarE; banks 2,3 in a second 2-bank tile copied by VectorE
            # as one wide op (measured slightly cheaper than 2x[128,512]).
            # The two pairs free independently, so the PE is released at
            # half-plane granularity.
            img = plane_view[pl]
            if wb == 0:
                pair_ps["a0"] = psa_pool.tile(
                    [P, 2 * W], mybir.dt.float32, tag="psa0", name=f"psa{pl}_01"
                )
            if wb == 2:
                pair_ps["a1"] = psa_pool.tile(
                    [P, 2 * W], mybir.dt.float32, tag="psa1", name=f"psa{pl}_23"
                )
            ps = pair_ps["a0"] if wb < 2 else pair_ps["a1"]
            view = ps[:, (wb % 2) * W : (wb % 2 + 1) * W]
            _emit_bank(
                nc,
                view,
                band,
                lambda kb: img[:, kb * W + wb * P : kb * W + (wb + 1) * P],
                last_bank=True,
            )
            if wb == 1:
                nc.scalar.copy(mids[pl][:, 0 : 2 * W], ps[:])
            elif wb == 3:
                nc.vector.tensor_copy(mids[pl][:, 2 * W : 4 * W], ps[:])

        def emit_b_bank(pl, mid, out2, hb2, pair_ps):
            # pass B bank: out2[:, hb2] = horizontal 5-sum of mid, transposed
            if hb2 == 0:
                pair_ps["b0"] = psb_pool.tile(
                    [P, 2 * W], mybir.dt.float32, tag="psb0", name=f"psb{pl}_01"
                )
            if hb2 == 2:
                pair_ps["b1"] = psb_pool.tile(
                    [P, 2 * W], mybir.dt.float32, tag="psb1", name=f"psb{pl}_23"
                )
            ps = pair_ps["b0"] if hb2 < 2 else pair_ps["b1"]
            view = ps[:, (hb2 % 2) * W : (hb2 % 2 + 1) * W]
            _emit_bank(
                nc,
                view,
                band,
                lambda kb: mid[:, kb * W + hb2 * P : kb * W + (hb2 + 1) * P],
                last_bank=True,
            )
            if hb2 == 1:
                nc.scalar.mul(out2[:, 0 : 2 * W], ps[:], scale)
            elif hb2 == 3:
                nc.vector.tensor_scalar_mul(out2[:, 2 * W : 4 * W], ps[:], scale)

        def emit_store(pl, out2):
            # One dense output DMA per plane on the sync HWDGE ring.
            nc.sync.dma_start(
                ys[pl],
                out2[:].rearrange("p (b w) -> p b w", w=W),
            )

        # Software pipeline, LAG planes deep: the PE stream interleaves
        # pass A of plane pl with pass B of plane pl-LAG at bank
        # granularity, so the PE never sits behind the PSUM->SBUF copies
        # it just queued.
        LAG = 1
        mids, outs = {}, {}
        mids[0] = mid_pool.tile([P, 4 * W], MM_DT, tag="mid", name="mid0")
        for pl in range(PLANES_PER_CORE + LAG):
            bp = pl - LAG
            if bp >= 0:
                outs[bp] = out_pool.tile(
                    [P, 4 * W], MM_DT, tag="out", name=f"out{bp}"
                )
            pair_ps = {}
            for b in range(4):
                if pl < PLANES_PER_CORE:
                    emit_a_bank(pl, b, pair_ps)
                if bp >= 0:
                    emit_b_bank(bp, mids[bp], outs[bp], b, pair_ps)
            if bp >= 0:
                emit_store(bp, outs[bp])
            if pl + 1 < PLANES_PER_CORE:
                mids[pl + 1] = mid_pool.tile(
                    [P, 4 * W], MM_DT, tag="mid", name=f"mid{pl + 1}"
                )

    nc.compile()
    return nc


_CACHE: dict = {}


def _get_nc(scale: float):
    if scale not in _CACHE:
        _CACHE[scale] = _build_nc(scale)
    return _CACHE[scale]


def kernel(x: np.ndarray, weight: np.ndarray, _trace: bool = False):
    x = np.ascontiguousarray(x, dtype=np.float32)
    w = np.asarray(weight, dtype=np.float32).reshape(KTAP, KTAP)
    scale = float(w[KPAD, KPAD])  # 1/25 for the box kernel

    # Host-pack: [pl, h, w] -> [pl, p, hb, w] with h = hb*128 + p, so
    # each partition line of an input DMA is a contiguous 4 KiB HBM
    # chunk per plane.
    xs = (
        x.reshape(PLANES_TOTAL, 4, P, W)
        .transpose(0, 2, 1, 3)
        .astype(NP_IO_DT)
    )
    xs = np.ascontiguousarray(xs)
    band = _band_host()

    nc = _get_nc(scale)
    in_maps = [
        {
            "xs": xs[k * PLANES_PER_CORE : (k + 1) * PLANES_PER_CORE],
            "band": band,
        }
        for k in range(N_CORES)
    ]
    res = run_bass_kernel_spmd(nc, in_maps, list(range(N_CORES)), trace=_trace)
    # ys[pl, p, hb, w] holds out[h = hb*128 + p, w]: invert on host.
    ys = np.concatenate(
        [np.asarray(r["ys"], dtype=np.float32) for r in res.results], axis=0
    )
    out = ys.transpose(0, 2, 1, 3).reshape(PLANES_TOTAL, H, W)
    if _trace:
        kernel.last_exec_time_ns = res.exec_time_ns
    return np.ascontiguousarray(out).reshape(16, 8, H, W)


# revision 20
# speedup vs baseline: 1.3820x; 1.0224x over previous
"""Depthwise 5x5 box filter (stride 1, 'same' zero padding) on TRN2.

Input x: (16, 8, 512, 512) f32, weight: (1, 1, 5, 5) f32 (uniform box kernel).
Output: (16, 8, 512, 512) f32.

Strategy
--------
Data-parallel over the 128 independent (n, c) planes: 16 planes per core
across 8 cores.  Per plane, the separable 5-tap box filter runs entirely on
the TensorEngine as two "transposing" banded matmuls:

  pass A:  mid[w, h'] = sum_h  img[h, w] * Band[h, h']   (vertical 5-sum)
  pass B:  out[h, w'] = sum_w  mid[w, h'] * Band[w, w']  (horizontal 5-sum)

Each pass contracts over the partition dimension of its input, so the
output of each matmul comes out transposed — two passes restore the
original orientation with no explicit transpose ops.  Band is a 0/1
banded Toeplitz matrix (values exactly representable), the final x(1/25)
scale is folded into the pass-B PSUM->SBUF copies.

Contraction over a full 512-row dimension is tiled into 4 K-blocks of
128; their overlapping 130/132-wide output windows accumulate in one
PSUM bank using the per-element has_written mechanism (verified on HW).

Host-side, the image is cast to fp16 (and results returned from fp16):
halves DMA traffic, and fp16 matmuls stream at 1 column/cycle on the PE.
HBM layouts are host-packed so DMAs have contiguous multi-KiB partition
lines:
  input  xs[pl, p, hb, w]  (p = h % 128; planes 0-1 get single-plane
                            DMAs for fast pipeline start, then 2/DMA)
  output ys[pl, p, hb, w]  (1 plane per DMA; pass B restores
                            orientation so packing matches input)

The kernel is HBM-bound: 16.8 MB of traffic at ~358 GB/s/core ≈ 47 us.
Scheduling decisions that keep HBM saturated end-to-end:
  - ALL input DMAs are HWDGE on the *scalar* (ACT) ring, issued up-front
    before any ACT copy work, into an 8 x 2-plane img pool (all 16
    planes SBUF-resident), so the input stream runs at full rate with
    no compute-side gating and no FIFO entry ever blocks another.
  - Output DMAs are HWDGE on the *sync* (SP) ring — a different physical
    ring, so input and output transfers round-robin fairly.  SWDGE
    (GpSimd) is deliberately NOT used for outputs: VectorE copy /
    tensor_scalar ops enter 2-port SBUF perf modes that lock GpSimd out
    of its shared SBUF port and starve SWDGE descriptor generation
    (observed: output stream capped at ~140 GB/s).  HWDGE descriptor
    generation is RTL and immune.
  - PSUM->SBUF copies are the compute-cadence limiter (~2.2-2.5
    us/plane/engine): per pass, ScalarE copies PSUM banks 0-1 as one
    [128,1024] op, VectorE copies banks 2-3 as two [128,512] ops
    (VectorE pays a bank-crossing penalty on wider PSUM reads).  Copies
    are emitted per-bank so they start as soon as their accumulation
    group finishes, keeping the PE from stalling on PSUM reuse (a single
    4-bank PSUM tile per pass was measured 1.2 us/plane slower).
"""

from contextlib import ExitStack

import numpy as np

import concourse.bacc as bacc
import concourse.tile as tile
from concourse import mybir
from concourse.bass_utils import run_bass_kernel_spmd

N_CORES = 8
PLANES_TOTAL = 128  # 16 batch * 8 channels
PLANES_PER_CORE = PLANES_TOTAL // N_CORES  # 16
H = W = 512
P = 128  # partitions / K-block
NB = P + 4  # band matrix columns
KTAP = 5
KPAD = 2

MM_DT = mybir.dt.float16
NP_IO_DT = np.float16

# Per PSUM bank (one 512-wide output window) the 4 K-block matmuls write
# overlapping band windows; the first (start=True) clears the whole-bank
# pending-zero region, and subsequent matmuls accumulate where written /
# overwrite where pending, per-element (PSUM has_written semantics).
# (kb, out_lo, out_hi, band_lo, band_hi, start)
BANK_PLAN = [
    (0, 0, 130, 2, 132, True),
    (1, 126, 258, 0, 132, False),
    (2, 254, 386, 0, 132, False),
    (3, 382, 512, 0, 130, False),
]


def _band_host() -> np.ndarray:
    """B[p, j] = 1.0 iff 0 <= j - p <= 4, shape [128, 132]."""
    b = np.zeros((P, NB), dtype=np.float32)
    for p in range(P):
        b[p, p : p + KTAP] = 1.0
    return b.astype(np.float16)


def _emit_bank(nc, ps, band, lhsT_of, last_bank):
    for i, (kb, o0, o1, b0, b1, start) in enumerate(BANK_PLAN):
        nc.tensor.matmul(
            ps[:, o0:o1],
            lhsT_of(kb),
            band[:, b0:b1],
            start=start,
            stop=(last_bank and i == len(BANK_PLAN) - 1),
        )


def _build_nc(scale: float):
    nc = bacc.Bacc("TRN2", num_devices=N_CORES, num_swdge_queues=1)
    xs = nc.declare_dram_parameter(
        "xs", [PLANES_PER_CORE, P, 4, W], MM_DT, isOutput=False
    )
    band_d = nc.declare_dram_parameter("band", [P, NB], MM_DT, isOutput=False)
    ys = nc.declare_dram_parameter(
        "ys", [PLANES_PER_CORE, P, 4, W], MM_DT, isOutput=True
    )

    with ExitStack() as ctx:
        tc = ctx.enter_context(tile.TileContext(nc))
        const_pool = ctx.enter_context(tc.tile_pool(name="const", bufs=1))
        img_pool = ctx.enter_context(tc.tile_pool(name="img", bufs=2))
        img2_pool = ctx.enter_context(tc.tile_pool(name="img2", bufs=7))
        mid_pool = ctx.enter_context(tc.tile_pool(name="mid", bufs=4))
        out_pool = ctx.enter_context(tc.tile_pool(name="out", bufs=8))
        psa_pool = ctx.enter_context(tc.tile_pool(name="psa", bufs=1, space="PSUM"))
        psb_pool = ctx.enter_context(tc.tile_pool(name="psb", bufs=1, space="PSUM"))

        band = const_pool.tile([P, NB], MM_DT, tag="band")
        nc.scalar.dma_start(band[:], band_d[:])

        # Input DMAs go on the scalar HWDGE ring (separate ring from the
        # sync-ring outputs).  Planes 0 and 1 get single-plane DMAs so
        # the first pass-A matmuls start ~1.5us earlier; the rest go 2
        # planes per DMA.  Only the first 4 groups (6 planes) are issued
        # up-front; the rest are issued from inside the plane loop ~6
        # planes ahead.  Pacing matters beyond buffering: Tile tracks DMA
        # completion on 8 round-robin semaphore lanes shared by all DMAs,
        # and a lane is held from issue to transfer completion — issuing
        # every input at t=0 makes output DMA k's lane wait on input k+5's
        # transfer (measured: first output issue pushed from ~18us to
        # ~23us and the whole output stream cadence-locked to it).
        groups = [[0], [1]] + [[i, i + 1] for i in range(2, PLANES_PER_CORE, 2)]
        plane_view = {}

        def emit_load(gi):
            grp = groups[gi]
            n = len(grp)
            pool = img_pool if n == 1 else img2_pool
            img = pool.tile(
                [P, n * 4 * W], MM_DT, tag=f"img{n}", name=f"img_g{gi}"
            )
            nc.scalar.dma_start(
                img[:].rearrange("p (g b w) -> p g b w", b=4, w=W),
                xs[grp[0] : grp[0] + n].rearrange("g p b w -> p g b w"),
            )
            for j, pl in enumerate(grp):
                plane_view[pl] = img[:, j * 4 * W : (j + 1) * 4 * W]

        for gi in range(4):
            emit_load(gi)

        def emit_a_bank(pl, wb, pair_ps):
            # pass A bank: mid[:, wb] = vertical 5-sum of img, transposed.
            # Banks 0,1 accumulate in one 2-bank psum tile copied by
            # ScalarE; banks 2,3 in a second 2-bank tile copied by VectorE
            # as one wide op (measured slightly cheaper than 2x[128,512]).
            # The two pairs free independently, so the PE is released at
            # half-plane granularity.
            img = plane_view[pl]
            if wb == 0:
                pair_ps["a0"] = psa_pool.tile(
                    [P, 2 * W], mybir.dt.float32, tag="psa0", name=f"psa{pl}_01"
                )
            if wb == 2:
                pair_ps["a1"] = psa_pool.tile(
                    [P, 2 * W], mybir.dt.float32, tag="psa1", name=f"psa{pl}_23"
                )
            ps = pair_ps["a0"] if wb < 2 else pair_ps["a1"]
            view = ps[:, (wb % 2) * W : (wb % 2 + 1) * W]
            _emit_bank(
                nc,
                view,
                band,
                lambda kb: img[:, kb * W + wb * P : kb * W + (wb + 1) * P],
                last_bank=True,
            )
            if wb == 1:
                nc.scalar.copy(mids[pl][:, 0 : 2 * W], ps[:])
            elif wb == 3:
                nc.vector.tensor_copy(mids[pl][:, 2 * W : 4 * W], ps[:])

        def emit_b_bank(pl, mid, out2, hb2, pair_ps):
            # pass B bank: out2[:, hb2] = horizontal 5-sum of mid, transposed
            if hb2 == 0:
                pair_ps["b0"] = psb_pool.tile(
                    [P, 2 * W], mybir.dt.float32, tag="psb0", name=f"psb{pl}_01"
                )
            if hb2 == 2:
                pair_ps["b1"] = psb_pool.tile(
                    [P, 2 * W], mybir.dt.float32, tag="psb1", name=f"psb{pl}_23"
                )
            ps = pair_ps["b0"] if hb2 < 2 else pair_ps["b1"]
            view = ps[:, (hb2 % 2) * W : (hb2 % 2 + 1) * W]
            _emit_bank(
                nc,
                view,
                band,
                lambda kb: mid[:, kb * W + hb2 * P : kb * W + (hb2 + 1) * P],
                last_bank=True,
            )
            if hb2 == 1:
                nc.scalar.mul(out2[:, 0 : 2 * W], ps[:], scale)
            elif hb2 == 3:
                nc.vector.tensor_scalar_mul(out2[:, 2 * W : 4 * W], ps[:], scale)

        def emit_store(pl, out2):
            # One dense output DMA per plane on the sync HWDGE ring.
            nc.sync.dma_start(
                ys[pl],
                out2[:].rearrange("p (b w) -> p b w", w=W),
            )

        # Software pipeline, LAG planes deep: the PE stream interleaves
        # pass A of plane pl with pass B of plane pl-LAG at bank
        # granularity, so the PE never sits behind the PSUM->SBUF copies
        # it just queued.
        LAG = 1
        mids, outs = {}, {}
        mids[0] = mid_pool.tile([P, 4 * W], MM_DT, tag="mid", name="mid0")
        for pl in range(PLANES_PER_CORE + LAG):
            # Paced input issues: group gi covers planes 2*gi-2, 2*gi-1;
            # issue it ~6 planes ahead of first use.
            if pl % 2 == 0 and 4 + pl // 2 < len(groups):
                emit_load(4 + pl // 2)
            bp = pl - LAG
            if bp >= 0:
                outs[bp] = out_pool.tile(
                    [P, 4 * W], MM_DT, tag="out", name=f"out{bp}"
                )
            pair_ps = {}
            for b in range(4):
                if pl < PLANES_PER_CORE:
                    emit_a_bank(pl, b, pair_ps)
                if bp >= 0:
                    emit_b_bank(bp, mids[bp], outs[bp], b, pair_ps)
            if bp >= 0:
                emit_store(bp, outs[bp])
            if pl + 1 < PLANES_PER_CORE:
                mids[pl + 1] = mid_pool.tile(
                    [P, 4 * W], MM_DT, tag="mid", name=f"mid{pl + 1}"
                )

    nc.compile()
    return nc


_CACHE: dict = {}


def _get_nc(scale: float):
    if scale not in _CACHE:
        _CACHE[scale] = _build_nc(scale)
    return _CACHE[scale]


def kernel(x: np.ndarray, weight: np.ndarray, _trace: bool = False):
    x = np.ascontiguousarray(x, dtype=np.float32)
    w = np.asarray(weight, dtype=np.float32).reshape(KTAP, KTAP)
    scale = float(w[KPAD, KPAD])  # 1/25 for the box kernel

    # Host-pack: [pl, h, w] -> [pl, p, hb, w] with h = hb*128 + p, so
    # each partition line of an input DMA is a contiguous 4 KiB HBM
    # chunk per plane.
    xs = (
        x.reshape(PLANES_TOTAL, 4, P, W)
        .transpose(0, 2, 1, 3)
        .astype(NP_IO_DT)
    )
    xs = np.ascontiguousarray(xs)
    band = _band_host()

    nc = _get_nc(scale)
    in_maps = [
        {
            "xs": xs[k * PLANES_PER_CORE : (k + 1) * PLANES_PER_CORE],
            "band": band,
        }
        for k in range(N_CORES)
    ]
    res = run_bass_kernel_spmd(nc, in_maps, list(range(N_CORES)), trace=_trace)
    # ys[pl, p, hb, w] holds out[h = hb*128 + p, w]: invert on host.
    ys = np.concatenate(
        [np.asarray(r["ys"], dtype=np.float32) for r in res.results], axis=0
    )
    out = ys.transpose(0, 2, 1, 3).reshape(PLANES_TOTAL, H, W)
    if _trace:
        kernel.last_exec_time_ns = res.exec_time_ns
    return np.ascontiguousarray(out).reshape(16, 8, H, W)


# revision 27
# speedup vs baseline: 1.4273x; 1.0328x over previous
"""Depthwise 5x5 box filter (stride 1, 'same' zero padding) on TRN2.

Input x: (16, 8, 512, 512) f32, weight: (1, 1, 5, 5) f32 (uniform box kernel).
Output: (16, 8, 512, 512) f32.

Strategy
--------
Data-parallel over the 128 independent (n, c) planes: 16 planes per core
across 8 cores.  Per plane, the separable 5-tap box filter runs entirely on
the TensorEngine as two "transposing" banded matmuls:

  pass A:  mid[w, h'] = sum_h  img[h, w] * Band[h, h']   (vertical 5-sum)
  pass B:  out[h, w'] = sum_w  mid[w, h'] * Band[w, w']  (horizontal 5-sum)

Each pass contracts over the partition dimension of its input, so the
output of each matmul comes out transposed — two passes restore the
original orientation with no explicit transpose ops.  Band is a 0/1
banded Toeplitz matrix (values exactly representable), the final x(1/25)
scale is folded into the pass-B PSUM->SBUF copies.

Contraction over a full 512-row dimension is tiled into 4 K-blocks of
128; their overlapping 130/132-wide output windows accumulate in one
PSUM bank using the per-element has_written mechanism (verified on HW).

Host-side, the image is cast to fp16 (and results returned from fp16):
halves DMA traffic, and fp16 matmuls stream at 1 column/cycle on the PE.
HBM layouts are host-packed so DMAs have contiguous multi-KiB partition
lines:
  input  xs[pl, p, hb, w]  (p = h % 128; planes 0-1 get single-plane
                            DMAs for fast pipeline start, then 2/DMA)
  output ys[pl, p, hb, w]  (1 plane per DMA; pass B restores
                            orientation so packing matches input)

The kernel is HBM-bound: 16.8 MB of traffic at ~358 GB/s/core ≈ 47 us.
Scheduling decisions that keep HBM saturated end-to-end:
  - ALL input DMAs are HWDGE on the *scalar* (ACT) ring, issued up-front
    before any ACT copy work, into an 8 x 2-plane img pool (all 16
    planes SBUF-resident), so the input stream runs at full rate with
    no compute-side gating and no FIFO entry ever blocks another.
  - Output DMAs are HWDGE on the *sync* (SP) ring — a different physical
    ring, so input and output transfers round-robin fairly.  SWDGE
    (GpSimd) is deliberately NOT used for outputs: VectorE copy /
    tensor_scalar ops enter 2-port SBUF perf modes that lock GpSimd out
    of its shared SBUF port and starve SWDGE descriptor generation
    (observed: output stream capped at ~140 GB/s).  HWDGE descriptor
    generation is RTL and immune.
  - PSUM->SBUF copies are the compute-cadence limiter (~2.2-2.5
    us/plane/engine): per pass, ScalarE copies PSUM banks 0-1 as one
    [128,1024] op, VectorE copies banks 2-3 as two [128,512] ops
    (VectorE pays a bank-crossing penalty on wider PSUM reads).  Copies
    are emitted per-bank so they start as soon as their accumulation
    group finishes, keeping the PE from stalling on PSUM reuse (a single
    4-bank PSUM tile per pass was measured 1.2 us/plane slower).
"""

from contextlib import ExitStack

import numpy as np

import concourse.bacc as bacc
import concourse.tile as tile
from concourse import mybir
from concourse.bass_utils import run_bass_kernel_spmd

N_CORES = 8
PLANES_TOTAL = 128  # 16 batch * 8 channels
PLANES_PER_CORE = PLANES_TOTAL // N_CORES  # 16
H = W = 512
P = 128  # partitions / K-block
NB = P + 4  # band matrix columns
KTAP = 5
KPAD = 2

MM_DT = mybir.dt.float16
NP_IO_DT = np.float16

# Per PSUM bank (one 512-wide output window) the 4 K-block matmuls write
# overlapping band windows; the first (start=True) clears the whole-bank
# pending-zero region, and subsequent matmuls accumulate where written /
# overwrite where pending, per-element (PSUM has_written semantics).
# (kb, out_lo, out_hi, band_lo, band_hi, start)
BANK_PLAN = [
    (0, 0, 130, 2, 132, True),
    (1, 126, 258, 0, 132, False),
    (2, 254, 386, 0, 132, False),
    (3, 382, 512, 0, 130, False),
]


def _band_host() -> np.ndarray:
    """B[p, j] = 1.0 iff 0 <= j - p <= 4, shape [128, 132]."""
    b = np.zeros((P, NB), dtype=np.float32)
    for p in range(P):
        b[p, p : p + KTAP] = 1.0
    return b.astype(np.float16)


def _emit_bank(nc, ps, band, lhsT_of, last_bank):
    for i, (kb, o0, o1, b0, b1, start) in enumerate(BANK_PLAN):
        nc.tensor.matmul(
            ps[:, o0:o1],
            lhsT_of(kb),
            band[:, b0:b1],
            start=start,
            stop=(last_bank and i == len(BANK_PLAN) - 1),
        )


def _build_nc(scale: float):
    nc = bacc.Bacc("TRN2", num_devices=N_CORES, num_swdge_queues=1)
    xs = nc.declare_dram_parameter(
        "xs", [PLANES_PER_CORE, P, 4, W], MM_DT, isOutput=False
    )
    band_d = nc.declare_dram_parameter("band", [P, NB], MM_DT, isOutput=False)
    ys = nc.declare_dram_parameter(
        "ys", [PLANES_PER_CORE, P, 4, W], MM_DT, isOutput=True
    )

    with ExitStack() as ctx:
        tc = ctx.enter_context(tile.TileContext(nc))
        const_pool = ctx.enter_context(tc.tile_pool(name="const", bufs=1))
        img_pool = ctx.enter_context(tc.tile_pool(name="img", bufs=2))
        img2_pool = ctx.enter_context(tc.tile_pool(name="img2", bufs=7))
        mid_pool = ctx.enter_context(tc.tile_pool(name="mid", bufs=4))
        out_pool = ctx.enter_context(tc.tile_pool(name="out", bufs=8))
        psa_pool = ctx.enter_context(tc.tile_pool(name="psa", bufs=1, space="PSUM"))
        psb_pool = ctx.enter_context(tc.tile_pool(name="psb", bufs=1, space="PSUM"))

        band = const_pool.tile([P, NB], MM_DT, tag="band")
        nc.scalar.dma_start(band[:], band_d[:])

        # Input DMAs go on the scalar HWDGE ring (separate ring from the
        # sync-ring outputs), all issued up-front: every plane has a
        # buffer, so no issue ever blocks, and the scalar sequencer is
        # done issuing before its first pass-B mul is needed.  Planes 0
        # and 1 get single-plane DMAs so the first pass-A matmuls start
        # ~1.5us earlier; the rest go 2 planes per DMA.
        groups = [[0], [1]] + [[i, i + 1] for i in range(2, PLANES_PER_CORE, 2)]
        plane_view = {}

        def emit_load(gi):
            grp = groups[gi]
            n = len(grp)
            pool = img_pool if n == 1 else img2_pool
            img = pool.tile(
                [P, n * 4 * W], MM_DT, tag=f"img{n}", name=f"img_g{gi}"
            )
            nc.scalar.dma_start(
                img[:].rearrange("p (g b w) -> p g b w", b=4, w=W),
                xs[grp[0] : grp[0] + n].rearrange("g p b w -> p g b w"),
            )
            for j, pl in enumerate(grp):
                plane_view[pl] = img[:, j * 4 * W : (j + 1) * 4 * W]

        for gi in range(len(groups)):
            emit_load(gi)

        def emit_a_bank(pl, wb, pair_ps):
            # pass A bank: mid[:, wb] = vertical 5-sum of img, transposed.
            # Banks 0,1 accumulate in one 2-bank psum tile copied by
            # ScalarE; banks 2,3 in a second 2-bank tile copied by VectorE
            # as one wide op (measured slightly cheaper than 2x[128,512]).
            # The two pairs free independently, so the PE is released at
            # half-plane granularity.
            img = plane_view[pl]
            if wb == 0:
                pair_ps["a0"] = psa_pool.tile(
                    [P, 2 * W], mybir.dt.float32, tag="psa0", name=f"psa{pl}_01"
                )
            if wb == 2:
                pair_ps["a1"] = psa_pool.tile(
                    [P, 2 * W], mybir.dt.float32, tag="psa1", name=f"psa{pl}_23"
                )
            ps = pair_ps["a0"] if wb < 2 else pair_ps["a1"]
            view = ps[:, (wb % 2) * W : (wb % 2 + 1) * W]
            _emit_bank(
                nc,
                view,
                band,
                lambda kb: img[:, kb * W + wb * P : kb * W + (wb + 1) * P],
                last_bank=True,
            )
            # All mid copies on VectorE: pass-B (out) copies then never
            # queue behind pass-A copies in an engine FIFO, and each tile
            # has a single writer (tighter semaphore chains).
            if wb == 1:
                nc.vector.tensor_copy(mids[pl][:, 0 : 2 * W], ps[:])
            elif wb == 3:
                nc.vector.tensor_copy(mids[pl][:, 2 * W : 4 * W], ps[:])

        def emit_b_bank(pl, mid, out2, hb2, pair_ps):
            # pass B bank: out2[:, hb2] = horizontal 5-sum of mid, transposed
            if hb2 == 0:
                pair_ps["b0"] = psb_pool.tile(
                    [P, 2 * W], mybir.dt.float32, tag="psb0", name=f"psb{pl}_01"
                )
            if hb2 == 2:
                pair_ps["b1"] = psb_pool.tile(
                    [P, 2 * W], mybir.dt.float32, tag="psb1", name=f"psb{pl}_23"
                )
            ps = pair_ps["b0"] if hb2 < 2 else pair_ps["b1"]
            view = ps[:, (hb2 % 2) * W : (hb2 % 2 + 1) * W]
            _emit_bank(
                nc,
                view,
                band,
                lambda kb: mid[:, kb * W + hb2 * P : kb * W + (hb2 + 1) * P],
                last_bank=True,
            )
            # All out copies on ScalarE; each half of the plane leaves as
            # its own output DMA as soon as its mul lands.
            if hb2 == 1:
                nc.scalar.mul(out2[:, 0 : 2 * W], ps[:], scale)
                emit_store_half(pl, out2, 0)
            elif hb2 == 3:
                nc.scalar.mul(out2[:, 2 * W : 4 * W], ps[:], scale)
                emit_store_half(pl, out2, 1)

        def emit_store_half(pl, out2, h):
            # Half-plane output DMAs on the sync HWDGE ring: bank pair 01
            # leaves while banks 23 are still being summed.
            nc.sync.dma_start(
                ys[pl, :, 2 * h : 2 * h + 2],
                out2[:, 2 * h * W : 2 * (h + 1) * W].rearrange(
                    "p (b w) -> p b w", w=W
                ),
            )

        # Software pipeline, LAG planes deep: the PE stream interleaves
        # pass A of plane pl with pass B of plane pl-LAG at bank
        # granularity, so the PE never sits behind the PSUM->SBUF copies
        # it just queued.
        LAG = 1
        mids, outs = {}, {}
        mids[0] = mid_pool.tile([P, 4 * W], MM_DT, tag="mid", name="mid0")
        for pl in range(PLANES_PER_CORE + LAG):
            bp = pl - LAG
            if bp >= 0:
                outs[bp] = out_pool.tile(
                    [P, 4 * W], MM_DT, tag="out", name=f"out{bp}"
                )
            pair_ps = {}
            for b in range(4):
                if pl < PLANES_PER_CORE:
                    emit_a_bank(pl, b, pair_ps)
                if bp >= 0:
                    emit_b_bank(bp, mids[bp], outs[bp], b, pair_ps)
            if pl + 1 < PLANES_PER_CORE:
                mids[pl + 1] = mid_pool.tile(
                    [P, 4 * W], MM_DT, tag="mid", name=f"mid{pl + 1}"
                )

    nc.compile()
    return nc


_CACHE: dict = {}


def _get_nc(scale: float):
    if scale not in _CACHE:
        _CACHE[scale] = _build_nc(scale)
    return _CACHE[scale]


def kernel(x: np.ndarray, weight: np.ndarray, _trace: bool = False):
    x = np.ascontiguousarray(x, dtype=np.float32)
    w = np.asarray(weight, dtype=np.float32).reshape(KTAP, KTAP)
    scale = float(w[KPAD, KPAD])  # 1/25 for the box kernel

    # Host-pack: [pl, h, w] -> [pl, p, hb, w] with h = hb*128 + p, so
    # each partition line of an input DMA is a contiguous 4 KiB HBM
    # chunk per plane.
    xs = (
        x.reshape(PLANES_TOTAL, 4, P, W)
        .transpose(0, 2, 1, 3)
        .astype(NP_IO_DT)
    )
    xs = np.ascontiguousarray(xs)
    band = _band_host()

    nc = _get_nc(scale)
    in_maps = [
        {
            "xs": xs[k * PLANES_PER_CORE : (k + 1) * PLANES_PER_CORE],
            "band": band,
        }
        for k in range(N_CORES)
    ]
    res = run_bass_kernel_spmd(nc, in_maps, list(range(N_CORES)), trace=_trace)
    # ys[pl, p, hb, w] holds out[h = hb*128 + p, w]: invert on host.
    ys = np.concatenate(
        [np.asarray(r["ys"], dtype=np.float32) for r in res.results], axis=0
    )
    out = ys.transpose(0, 2, 1, 3).reshape(PLANES_TOTAL, H, W)
    if _trace:
        kernel.last_exec_time_ns = res.exec_time_ns
    return np.ascontiguousarray(out).reshape(16, 8, H, W)
